# revision 14
# baseline (speedup 1.0000x reference)
"""Mean-aggregator (GNN message passing) Bass kernel for 8 trn2 NeuronCores.

Algorithm: out[s] = mean over edges e with seg_ids[e]==s of features[neigh_idx[e]].

Sharding: data-parallel over destination segments. Core c owns segments
[c*5120, (c+1)*5120) = 40 aligned blocks of 128 segments. Since seg_ids is
sorted, each core's edges are a contiguous slice. All 8 cores run one
identical SPMD program; all data-dependent structure is padded host-side to
common sizes (maxima over all cores/blocks).

Gather: the f16 feature table is fetched edge-by-edge with the native
dma_gather instruction (256B rows). dma_gather indices are int16, so the
50000-row table is split at a host-tuned row SPLIT < 32768: each block's
edges are partitioned (A: node < SPLIT, B: node >= SPLIT), each section
padded to a fixed tile count (KA/KB tiles of 128 edges). One gather call per
(block, section), queues rotated so all four Q7 SWDGE pairs generate
descriptors concurrently. Pad slots carry idx = -1 (ucode skips trailing
negative indices: no descriptor, no HBM read) except on each X-buffer's
first use, where pads point at row 0 so every slot holds a finite f16
(skipped pad slots later read stale-but-finite data; relseg = -1 masks them
out of the matmul).

Compute per block: DVE builds S[e, s] = (relseg[e] == s) for the block's K
tiles in one tensor_tensor (iota compare); PE accumulates
  psum += S.T @ X      [128 segs, 128 feats]
over the K tiles, two blocks interleaved across two PSUM banks. The flush
runs on the ACT engine (activation Copy with per-partition scale =
reciprocal counts, host-precomputed) and the [128, 128] f32 block is DMAd
out on alternating HWDGE queues.
"""

import numpy as np

NUM_NODES = 50000
FEAT = 128
NUM_BATCH = 40000
N_CORES = 8
BLOCKS_PER_CORE = 40
SEG_BLOCK = 128
SEGS_PER_CORE = BLOCKS_PER_CORE * SEG_BLOCK  # 5120
XBUFS = 8  # X-tile pool depth (blocks in flight); also host pad-policy knob
SBUFS = 6  # S-tile pool depth
PREFETCH = 6  # blocks of gather/S-build issued ahead of the matmul loop

_program_cache: dict = {}


def _build_program(KA: int, KB: int, split: int, swap_tt: bool = False):
    """Build (and cache) the SPMD Bass program for KA/KB tiles per block."""
    key = ("v2", KA, KB, split, swap_tt)
    if key in _program_cache:
        return _program_cache[key]

    import concourse.bacc as bacc
    import concourse.bass as bass
    import concourse.mybir as mybir
    import concourse.tile as tile

    K = KA + KB
    T = BLOCKS_PER_CORE * K
    f32 = mybir.dt.float32
    f16 = mybir.dt.float16
    i16 = mybir.dt.int16

    i32 = mybir.dt.int32

    nc = bacc.Bacc(
        "TRN2", target_bir_lowering=False, debug=False, num_swdge_queues=4
    )
    feat = nc.dram_tensor("features", [NUM_NODES, FEAT], f16, kind="ExternalInput")
    # wrapped int16 gather indices, block-major: block b owns columns
    # [b*K*8, (b+1)*K*8): A-section first (KA*8 cols), then B (KB*8 cols).
    # Shipped as 16 partition rows; replicated to 128 on-chip.
    idxw = nc.dram_tensor("idxw", [16, T * 8], i16, kind="ExternalInput")
    # relseg[p, b*K + j] = relative segment of edge slot (p, tile j) of block b
    relseg = nc.dram_tensor("relseg", [128, T], f16, kind="ExternalInput")
    # rc[p, b] = 1/max(count, 1) for segment b*128+p of this core
    rc = nc.dram_tensor("rc", [128, BLOCKS_PER_CORE], f32, kind="ExternalInput")
    iotad = nc.dram_tensor("iotad", [128, 128], f16, kind="ExternalInput")
    # cnts[ci] = number of non-negative idxs of gather call ci (3 per block)
    cnts = nc.dram_tensor("cnts", [1, 3 * BLOCKS_PER_CORE], i32, kind="ExternalInput")
    out = nc.dram_tensor("out", [SEGS_PER_CORE, FEAT], f32, kind="ExternalOutput")

    with tile.TileContext(nc) as tc:
        with (
            tc.tile_pool(name="const", bufs=1) as constp,
            tc.tile_pool(name="idx", bufs=1) as idxp,
            tc.tile_pool(name="xa", bufs=XBUFS) as xap,
            tc.tile_pool(name="xb", bufs=XBUFS) as xbp,
            tc.tile_pool(name="s", bufs=SBUFS) as sp,
            tc.tile_pool(name="fl", bufs=4) as flp,
            tc.tile_pool(name="ps", bufs=3, space="PSUM") as pp,
        ):
            idxw_sb = idxp.tile([128, T * 8], i16)
            relseg_sb = idxp.tile([128, T], f16)
            rc_sb = idxp.tile([128, BLOCKS_PER_CORE], f32)
            iota_seed = constp.tile([128, 128], f16)
            iota_sb = constp.tile([128, K * 128], f16)
            cnts_sb = idxp.tile([128, 3 * BLOCKS_PER_CORE], i32)

            # Preloads. idxw ([16, cols], in 2 block-major stages) + its
            # 16->128 partition replication on the sync HWDGE queue; relseg/
            # iota/cnts/rc on the scalar (ACT) HWDGE queue in parallel.
            stage_cols = [0, 8 * K * 8, T * 8]  # blocks 0-7, then 8-39
            for s_ in range(2):
                c0, c1 = stage_cols[s_], stage_cols[s_ + 1]
                nc.sync.dma_start(idxw_sb[0:16, c0:c1], idxw[:, c0:c1])
                for p in (16, 32, 64):
                    nc.sync.dma_start(
                        idxw_sb[p : 2 * p, c0:c1], idxw_sb[0:p, c0:c1]
                    )
            half = (T // 2 // K) * K
            nc.scalar.dma_start(relseg_sb[:, :half], relseg[:, :half])
            nc.scalar.dma_start(relseg_sb[:, half:], relseg[:, half:])
            nc.scalar.dma_start(iota_seed[:], iotad[:])
            nc.scalar.dma_start(cnts_sb[0:1, :], cnts[:])
            nc.scalar.dma_start(rc_sb[:], rc[:])
            # broadcast the 128-col iota seed across the K tile columns by
            # doubling copies on DVE (idle during the preamble)
            nc.vector.tensor_copy(out=iota_sb[:, :128], in_=iota_seed[:])
            w = 128
            while w < K * 128:
                w2 = min(w, K * 128 - w)
                nc.vector.tensor_copy(
                    out=iota_sb[:, w : w + w2], in_=iota_sb[:, :w2]
                )
                w += w2

            sts: list = [None] * BLOCKS_PER_CORE
            xas: list = [None] * BLOCKS_PER_CORE
            xbs: list = [None] * BLOCKS_PER_CORE
            qctr = [0]
            # single count register, reused: the Pool sequencer captures the
            # value at dispatch, so the next reg_load cannot clobber an
            # in-flight gather's count
            cnt_reg = nc.gpsimd.alloc_register("gather_cnt")

            def build_s(b):
                st = sp.tile([128, K * 128], f16, tag="st")
                o = st[:].rearrange("p (j s) -> p j s", s=128)
                i0 = iota_sb[:].rearrange("p (j s) -> p j s", s=128)
                i1 = relseg_sb[:, b * K : (b + 1) * K].to_broadcast([128, K, 128])
                if swap_tt:
                    i0, i1 = i1, i0
                nc.vector.tensor_tensor(
                    out=o, in0=i0, in1=i1, op=mybir.AluOpType.is_equal
                )
                sts[b] = st

            KB1 = (KB + 1) // 2  # B gather split in two for queue balance

            def gather(b):
                # queue_num must track the tile scheduler's DMASW lane
                # round-robin, which follows scheduled instruction order —
                # keep creation order uniform (A, B1, B2 per block) so the
                # scheduler preserves it; 3 calls/block staggers the A/B
                # sizes across the 4 queues on its own. num_idxs_reg is
                # loaded per call from cnts (exact valid-idx count; trailing
                # -1 pads are skipped by the ucode: no descriptor, no read).
                xa = xap.tile([128, KA * 128], f16, tag="xa")
                xb = xbp.tile([128, KB * 128], f16, tag="xb")
                calls = [
                    (xa[:].rearrange("p (c e) -> p c e", e=128),
                     feat[:split, :], b * K * 8, KA),
                    (xb[:, : KB1 * 128].rearrange("p (c e) -> p c e", e=128),
                     feat[split:, :], b * K * 8 + KA * 8, KB1),
                    (xb[:, KB1 * 128 :].rearrange("p (c e) -> p c e", e=128),
                     feat[split:, :], b * K * 8 + (KA + KB1) * 8, KB - KB1),
                ]
                for out_ap, table, col0, ktiles in calls:
                    c = qctr[0]
                    qctr[0] += 1
                    nc.gpsimd.reg_load(cnt_reg, cnts_sb[0:1, c : c + 1])
                    nc.gpsimd.dma_gather(
                        out_ap=out_ap,
                        in_ap=table,
                        idxs_ap=idxw_sb[:, col0 : col0 + ktiles * 8],
                        num_idxs=ktiles * 128,
                        num_idxs_reg=cnt_reg,
                        elem_size=FEAT,
                        single_packet=False,
                        queue_num=c % 4,
                    )
                xas[b] = xa
                xbs[b] = xb

            def rhs(b, j):
                if j < KA:
                    return xas[b][:, j * 128 : (j + 1) * 128]
                return xbs[b][:, (j - KA) * 128 : (j - KA + 1) * 128]

            for b in range(PREFETCH):
                build_s(b)
            for b in range(PREFETCH):
                gather(b)

            for b0 in range(0, BLOCKS_PER_CORE, 2):
                for nb in (b0 + PREFETCH, b0 + PREFETCH + 1):
                    if nb < BLOCKS_PER_CORE:
                        build_s(nb)
                        gather(nb)
                pse = pp.tile([128, FEAT], f32, space="PSUM", tag="pse")
                pso = pp.tile([128, FEAT], f32, space="PSUM", tag="pso")
                for j in range(K):
                    for b, ps in ((b0, pse), (b0 + 1, pso)):
                        nc.tensor.matmul(
                            ps[:], lhsT=sts[b][:, j * 128 : (j + 1) * 128],
                            rhs=rhs(b, j),
                            start=(j == 0), stop=(j == K - 1),
                        )
                for b, ps in ((b0, pse), (b0 + 1, pso)):
                    ob = flp.tile([128, FEAT], f32, tag="ob")
                    nc.scalar.activation(
                        ob[:], ps[:], mybir.ActivationFunctionType.Copy,
                        scale=rc_sb[:, b : b + 1],
                    )
                    eng = nc.sync if b % 2 == 0 else nc.scalar
                    eng.dma_start(out[b * 128 : (b + 1) * 128, :], ob[:])
                    sts[b] = None
                    xas[b] = None
                    xbs[b] = None

    nc.compile()
    _program_cache[key] = nc
    return nc


def _prepare_inputs(features, neigh_idx, seg_ids):
    """Shard edges by segment block; within each block partition edges into
    A (node < split) then B, pad sections to KA/KB tiles. The split point is
    tuned to minimize total padded tiles. Returns (features f16, per-core
    idxw [128, T*8] i16 block-major, per-core relseg [128, T] f16, per-core
    rc [128, 40] f32, iota, KA, KB, split)."""
    n_blocks = N_CORES * BLOCKS_PER_CORE
    bases = np.arange(n_blocks + 1, dtype=np.int64) * SEG_BLOCK
    bnd = np.searchsorted(seg_ids, bases)

    nidx64 = np.asarray(neigh_idx)
    seg64 = np.asarray(seg_ids)

    # tune the table split point: minimize KA+KB over candidates
    lo = max(0, NUM_NODES - 32768)
    candidates = np.linspace(lo + 256, 32768, 12).astype(np.int64)
    block_nodes = [np.sort(nidx64[bnd[i] : bnd[i + 1]]) for i in range(n_blocks)]
    sizes = np.array([len(x) for x in block_nodes])
    best = None
    for s in candidates:
        na = np.array([np.searchsorted(x, s) for x in block_nodes])
        nb = sizes - na
        ka = max(1, -(-int(na.max()) // 128))
        kb = -(-int(nb.max()) // 128)
        if best is None or ka + kb < best[0] + best[1]:
            best = (ka, kb, int(s))
    KA, KB, split = best
    K = KA + KB
    T = BLOCKS_PER_CORE * K

    def wrap16(a):
        # flat i -> [i % 16, i // 16]
        return a.reshape(-1, 16).T

    KB1 = (KB + 1) // 2
    sec_tiles = [KA, KB1, KB - KB1]
    idxw = np.zeros((N_CORES, 16, T * 8), dtype=np.int16)
    relseg = np.full((N_CORES, 128, T), -1.0, dtype=np.float16)
    cnts = np.zeros((N_CORES, 3 * BLOCKS_PER_CORE), dtype=np.int32)
    for i in range(n_blocks):
        c, b = divmod(i, BLOCKS_PER_CORE)
        lo_, hi_ = bnd[i], bnd[i + 1]
        nodes = nidx64[lo_:hi_]
        rs = (seg64[lo_:hi_] - bases[i]).astype(np.float16)
        a_mask = nodes < split
        an, ar = nodes[a_mask], rs[a_mask]
        bn, br = (nodes[~a_mask] - split), rs[~a_mask]
        # per gather call: valid idxs first, then 0-pads up to `valid`
        # (first X-buffer use gathers full width so later skipped slots
        # read finite stale data; otherwise valid >= 256 so all 16 SDMA
        # engines get a descriptor), then -1 skip-pads
        secs = [(an, ar), (bn[: KB1 * 128], br[: KB1 * 128]),
                (bn[KB1 * 128 :], br[KB1 * 128 :])]
        col = b * K * 8
        j0 = b * K
        for s_, ((sn, sr), kt) in enumerate(zip(secs, sec_tiles)):
            w = kt * 128
            n = len(sn)
            valid = w if b < XBUFS else min(w, max(n, 256))
            flat = np.full(w, -1, np.int16)
            flat[:valid] = 0
            flat[:n] = sn.astype(np.int16)
            idxw[c, :, col : col + kt * 8] = wrap16(flat)
            rsec = np.full(w, -1.0, np.float16)
            rsec[:n] = sr
            relseg[c, :, j0 : j0 + kt] = rsec.reshape(kt, 128).T
            cnts[c, 3 * b + s_] = valid
            col += kt * 8
            j0 += kt

    counts = np.bincount(seg64, minlength=N_CORES * SEGS_PER_CORE).astype(np.float64)
    rcg = (1.0 / np.maximum(counts, 1.0)).astype(np.float32)
    rc = [
        np.ascontiguousarray(
            rcg[c * SEGS_PER_CORE : (c + 1) * SEGS_PER_CORE]
            .reshape(BLOCKS_PER_CORE, 128)
            .T
        )
        for c in range(N_CORES)
    ]
    feat16 = np.ascontiguousarray(features.astype(np.float16))
    iotad = np.tile(np.arange(128, dtype=np.float16)[None, :], (128, 1))
    idxw_l = [np.ascontiguousarray(idxw[c]) for c in range(N_CORES)]
    relseg_l = [np.ascontiguousarray(relseg[c]) for c in range(N_CORES)]
    cnts_l = [np.ascontiguousarray(cnts[c : c + 1]) for c in range(N_CORES)]
    return feat16, idxw_l, relseg_l, rc, iotad, cnts_l, KA, KB, split


LAST_RESULT = None


def _subprocess_fallback(features, neigh_idx, seg_ids, num_batch):
    """Re-run the whole kernel in a fresh process (clean device/PJRT state).
    Used only if in-process retries keep failing on a transient device
    fault. Guarded by an env var against recursion."""
    import os
    import subprocess
    import sys
    import tempfile

    kdir = os.path.dirname(os.path.abspath(__file__))
    with tempfile.TemporaryDirectory() as td:
        np.save(os.path.join(td, "features.npy"), np.asarray(features, np.float32))
        np.save(os.path.join(td, "neigh_idx.npy"), np.asarray(neigh_idx))
        np.save(os.path.join(td, "seg_ids.npy"), np.asarray(seg_ids))
        code = (
            "import sys, numpy as np\n"
            f"sys.path.insert(0, {kdir!r})\n"
            "import kernel\n"
            f"td = {td!r}\n"
            "out = kernel.kernel(\n"
            "    np.load(td + '/features.npy'),\n"
            "    np.load(td + '/neigh_idx.npy'),\n"
            "    np.load(td + '/seg_ids.npy'),\n"
            f"    {int(num_batch)},\n"
            ")\n"
            "np.save(td + '/out.npy', out)\n"
        )
        env = dict(os.environ, KERNEL_NO_SUBPROC="1")
        for attempt in range(3):
            p = subprocess.run(
                [sys.executable, "-c", code], env=env, timeout=1200,
                capture_output=True, text=True,
            )
            if p.returncode == 0:
                return np.load(os.path.join(td, "out.npy"))
        raise RuntimeError(
            f"kernel subprocess failed:\n{p.stdout[-2000:]}\n{p.stderr[-2000:]}"
        )


def kernel(features, neigh_idx, seg_ids, num_batch, _trace=False):
    global LAST_RESULT
    import os

    from concourse.bass_utils import run_bass_kernel_spmd

    features = np.asarray(features, dtype=np.float32)
    neigh_idx = np.asarray(neigh_idx)
    seg_ids = np.asarray(seg_ids)
    nb = int(num_batch)
    assert nb == NUM_BATCH, nb
    assert features.shape == (NUM_NODES, FEAT), features.shape

    feat16, idxw, relseg_t, rc, iotad, cnts, KA, KB, split = _prepare_inputs(
        features, neigh_idx, seg_ids
    )
    nc = _build_program(KA, KB, split)

    in_maps = [
        {
            "features": feat16,
            "idxw": idxw[c],
            "relseg": relseg_t[c],
            "rc": rc[c],
            "iotad": iotad,
            "cnts": cnts[c],
        }
        for c in range(N_CORES)
    ]
    res = None
    err = None
    for attempt in range(3):
        try:
            res = run_bass_kernel_spmd(
                nc,
                in_maps,
                core_ids=list(range(N_CORES)),
                trace=_trace and attempt == 0,
            )
            break
        except Exception as e:  # transient NRT faults: retry on clean state
            err = e
    if res is None:
        if os.environ.get("KERNEL_NO_SUBPROC"):
            raise err
        return _subprocess_fallback(features, neigh_idx, seg_ids, num_batch)
    LAST_RESULT = res

    out = np.empty((NUM_BATCH, FEAT), dtype=np.float32)
    for c in range(N_CORES):
        lo = c * SEGS_PER_CORE
        hi = min(lo + SEGS_PER_CORE, NUM_BATCH)
        if hi > lo:
            out[lo:hi] = res.results[c]["out"][: hi - lo]
    return out
